# revision 21
# baseline (speedup 1.0000x reference)
import sys
if '/opt/trn_rl_repo' not in sys.path:
    sys.path.insert(0, '/opt/trn_rl_repo')
import numpy as np
import ml_dtypes

import concourse.bass as bass
import concourse.bacc as bacc
import concourse.tile as tile
from concourse import mybir
from concourse.bass_utils import run_bass_kernel_spmd
from concourse.masks import make_identity

F32 = mybir.dt.float32
BF = mybir.dt.bfloat16
AF = mybir.ActivationFunctionType
MUL = mybir.AluOpType.mult
ADD = mybir.AluOpType.add
SUB = mybir.AluOpType.subtract
P = 128
D, H, DK, DV, NL = 768, 8, 64, 64, 2
B, LC, LQ, LK = 8, 512, 160, 512
DC = D // P      # 6 chunks of the 768 dim
CC = LC // P     # 4 chunks of the 512 token dim
QCH = [(0, 128), (128, 32)]   # (offset, size) chunks of LQ=160
SCALE = 0.125    # log_512(512)/sqrt(64)
EPS = 1e-6

_CACHE = {}
bf16 = ml_dtypes.bfloat16


def _build():
    nc = bacc.Bacc()
    dt = {}

    def din(name, shape, dtp=BF):
        dt[name] = nc.dram_tensor(name, list(shape), dtp, kind="ExternalInput")
        return dt[name]

    din('S_nat', (LC, D)); din('S_T', (D, LC))
    din('Q_nat', (2 * P, D)); din('Q_T', (D, LQ))     # Q_nat zero-padded to 256
    din('E_nat', (2 * P, D)); din('E_T', (D, LQ))
    din('KE_T', (D, LK))
    din('vecs_b', (D, 2))            # cols: w4C, w4Q (bf16)
    din('w4mlu_f', (D,), F32)
    din('cqa_b', (D,), F32)
    din('cqa_Wp', (12 * P, 2 * D))   # packed pairs of cqa_W.T row-chunks
    for l in range(NL):
        din(f'sQKV{l}', (3 * P, 2 * 3 * H * DK))  # [wq|wk|wv] chunk pairs
        din(f'sFC{l}', (2 * P, 2 * D))            # fc row-chunk pairs
        din(f'cQp{l}', (3 * P, 2 * H * DK))       # cWq row-chunk pairs
        din(f'cKV{l}', (6 * P, 4 * H * DK))       # [wk|wv] chunk pairs
        din(f'cFC{l}', (2 * P, 2 * D))
        din(f'ln{l}', (D, 4))        # cols: n1g, n1b, n2g, n2b
    out_t = nc.dram_tensor('out_t', [3 * D, LC], BF, kind="ExternalOutput")

    with tile.TileContext(nc) as tc:
        _emit(nc, tc, dt, out_t)
    nc.compile()
    return nc


def _emit(nc, tc, dt, out_t):
    from contextlib import ExitStack
    ctx = ExitStack()
    const = ctx.enter_context(tc.tile_pool(name="const", bufs=1))
    persist = ctx.enter_context(tc.tile_pool(name="persist", bufs=1))

    # round-robin DMA issue across sync+gpsimd queues (keep ACT free for
    # activations: DMA issue instructions would head-of-line block it)
    dma_engines = [nc.sync, nc.gpsimd]
    dma_rr = [0]

    def dma(out, in_):
        e = dma_engines[dma_rr[0] % len(dma_engines)]
        dma_rr[0] += 1
        e.dma_start(out=out, in_=in_)

    # att result tiles stay resident in SBUF for phase 2 (also DMA'd to out_t)
    att_T = [persist.tile([P, LC], BF, name=f"attT{i}") for i in range(2 * DC)]
    ke_T = persist.tile([P, DC, LK], BF, name="keT")

    # ---------------- input loads, critical-first, single-issue ----------------
    wpool = tc.alloc_tile_pool(name="wpool", bufs=1)
    s2q = tc.alloc_tile_pool(name="s2q", bufs=1)
    cqaw = tc.alloc_tile_pool(name="cqaw", bufs=1)
    qin = tc.alloc_tile_pool(name="qin", bufs=1)
    ps = ctx.enter_context(tc.tile_pool(name="ps", bufs=1, space="PSUM"))

    S_T = s2q.tile([P, DC, LC], BF, name="ST")
    dma(S_T, dt['S_T'].rearrange("(c p) n -> p c n", p=P))
    vecs = const.tile([P, DC, 2], BF)
    dma(vecs, dt['vecs_b'].rearrange("(c p) v -> p c v", p=P))
    w4mlu = const.tile([P, DC], F32)
    dma(w4mlu, dt['w4mlu_f'].rearrange("(c p) -> p c", p=P))
    qe_in = {}
    for tag, QN, QT in (("q", dt['Q_nat'], dt['Q_T']), ("e", dt['E_nat'], dt['E_T'])):
        Qn = qin.tile([P, 2, D], BF, name=f"Qn{tag}")
        dma(Qn, QN.rearrange("(i p) d -> p i d", p=P))
        Qt = qin.tile([P, DC, LQ], BF, name=f"Qt{tag}")
        dma(Qt, QT.rearrange("(c p) n -> p c n", p=P))
        qe_in[tag] = (Qn, Qt)
    S_nat = s2q.tile([P, CC, D], BF, name="Snat")
    dma(S_nat, dt['S_nat'].rearrange("(c p) d -> p c d", p=P))
    dma(ke_T, dt['KE_T'].rearrange("(c p) n -> p c n", p=P))
    cqab = const.tile([P, DC], F32)
    dma(cqab, dt['cqa_b'].rearrange("(c p) -> p c", p=P))
    cqa_Wp = cqaw.tile([P, 12, 2 * D], BF, name="cqaW")
    dma(cqa_Wp, dt['cqa_Wp'].rearrange("(k p) n -> p k n", p=P))

    def cqa_slice(k, mc):
        # lhsT [128, 128] for contraction chunk k (of 24) and out chunk mc
        return cqa_Wp[:, k // 2, (k % 2) * D + mc * P:(k % 2) * D + (mc + 1) * P]

    # ---------------- constants (after DMA issues) ----------------
    ident = const.tile([P, P], BF)
    make_identity(nc, ident)
    ones_row = const.tile([1, P], BF)
    nc.gpsimd.memset(ones_row, 1.0)
    ones_col = const.tile([P, 1], BF)
    nc.gpsimd.memset(ones_col, 1.0)
    ones_row_f = const.tile([1, P], F32)
    nc.gpsimd.memset(ones_row_f, 1.0)
    eps_t = const.tile([1, 1], F32)
    nc.gpsimd.memset(eps_t, EPS)
    scr = const.tile([1, 1], F32)
    nc.gpsimd.memset(scr, 1.0)
    nc.scalar.activation(scr, scr, AF.Exp)   # preload exp table during DMA wait
    # selN[p, 128*b + q] = 1 iff p == 32*(2b + q//64): broadcasts denominator
    # rows parked at partitions {0,32,64,96} onto the two 64-row head halves.
    selN = const.tile([P, 2 * P], BF)
    nc.gpsimd.memset(selN, 1.0)
    nc.gpsimd.affine_select(out=selN.rearrange("p (a q) -> p a q", q=DV),
                            in_=selN.rearrange("p (a q) -> p a q", q=DV),
                            compare_op=mybir.AluOpType.is_equal,
                            fill=0.0, base=0, pattern=[[-32, 4], [0, DV]],
                            channel_multiplier=1)

    # ---------------- phase-2 weight prefetch machinery ----------------
    def load_w(l, is_self):
        tag = ('s' if is_self else 'c') + str(l)
        w = {}
        if is_self:
            qkv = wpool.tile([P, 3, 2 * 3 * H * DK], BF, name=f"qkv{tag}",
                             tag="qkv", bufs=1)
            dma(qkv, dt[f'sQKV{l}'].rearrange("(j p) n -> p j n", p=P))
            w['q'] = lambda k, m: qkv[:, k // 2, (k % 2) * 1536 + m * P:
                                      (k % 2) * 1536 + (m + 1) * P]
            w['k'] = lambda k, m: qkv[:, k // 2, (k % 2) * 1536 + 512 + m * P:
                                      (k % 2) * 1536 + 512 + (m + 1) * P]
            w['v'] = lambda k: qkv[:, k // 2, (k % 2) * 1536 + 1024:
                                   (k % 2) * 1536 + 1536]
        else:
            qw = wpool.tile([P, 3, 2 * H * DK], BF, name=f"qw{tag}", tag="qw",
                            bufs=1)
            dma(qw, dt[f'cQp{l}'].rearrange("(j p) n -> p j n", p=P))
            kv = wpool.tile([P, 6, 4 * H * DK], BF, name=f"kvw{tag}", tag="kvw",
                            bufs=1)
            dma(kv, dt[f'cKV{l}'].rearrange("(j p) n -> p j n", p=P))
            w['q'] = lambda k, m: qw[:, k // 2, (k % 2) * 512 + m * P:
                                     (k % 2) * 512 + (m + 1) * P]
            w['k'] = lambda k, m: kv[:, k // 2, (k % 2) * 1024 + m * P:
                                     (k % 2) * 1024 + (m + 1) * P]
            w['v'] = lambda k: kv[:, k // 2, (k % 2) * 1024 + 512:
                                  (k % 2) * 1024 + 1024]
        wf = wpool.tile([P, 2, 2 * D], BF, name=f"wf{tag}",
                        tag=("sfc" if is_self else "cfc"), bufs=1)
        dma(wf, dt[('sFC' if is_self else 'cFC') + str(l)]
            .rearrange("(j p) n -> p j n", p=P))
        w['fc'] = lambda k, d: wf[:, k // 2, (k % 2) * D + d * P:
                                  (k % 2) * D + (d + 1) * P]
        return w

    w_s0 = load_w(0, True)
    w_c0 = load_w(0, False)

    # ---------------- phase 1: s2q twice ----------------
    cm_T = s2q.tile([P, DC, LC], BF, name="cmT")
    for d in range(DC):
        nc.vector.tensor_scalar_mul(cm_T[:, d, :], S_T[:, d, :], w4mlu[:, d:d + 1])
    # s0_row [1, LC]
    ps0 = ps.tile([1, LC], F32, tag="b", bufs=3)
    for d in range(DC):
        nc.tensor.matmul(ps0, vecs[:, d, 0:1], S_T[:, d, :], start=(d == 0),
                         stop=(d == DC - 1))
    s0_row = s2q.tile([1, LC], BF)
    nc.vector.tensor_copy(s0_row, ps0)

    s2qt = tc.alloc_tile_pool(name="s2qt", bufs=1)

    def s2q_call(tag, row0):
        Qn, Qt = qe_in[tag]
        po = s2qt
        # s1 [LQ,1]
        s1 = []
        for qi, (qo, qs) in enumerate(QCH):
            pq = ps.tile([P, 1], F32, tag="b", bufs=3)
            for d in range(DC):
                nc.tensor.matmul(pq[:qs], Qt[:, d, qo:qo + qs], vecs[:, d, 1:2],
                                 start=(d == 0), stop=(d == DC - 1))
            t = po.tile([P, 1], F32, name=f"s1{tag}{qi}", tag=f"s1{qi}", bufs=2)
            nc.vector.tensor_copy(t[:qs], pq[:qs])
            s1.append(t)
        # score_T + exp -> e_t, row sums -> r_t; e_t_norm
        e_t, etn, r_t = [], [], []
        for qi, (qo, qs) in enumerate(QCH):
            psc_t = ps.tile([P, LC], F32, tag="a", bufs=4)
            for d in range(DC):
                nc.tensor.matmul(psc_t[:qs], Qt[:, d, qo:qo + qs], cm_T[:, d, :],
                                 start=(d == 0), stop=False)
            nc.tensor.matmul(psc_t[:qs], ones_row[:1, :qs], s0_row,
                             start=False, stop=True)
            et = po.tile([P, LC], BF, name=f"et{tag}{qi}", tag=f"et{qi}", bufs=2)
            st = po.tile([P, 1], F32, name=f"st{tag}{qi}", tag=f"st{qi}", bufs=2)
            nc.scalar.activation(et[:qs], psc_t[:qs], AF.Exp, bias=s1[qi][:qs],
                                 scale=1.0, accum_out=st[:qs])
            rt = po.tile([P, 1], F32, name=f"rt{tag}{qi}", tag=f"rt{qi}", bufs=2)
            nc.vector.reciprocal_approx_fast(rt[:qs], st[:qs])
            en = po.tile([P, LC], BF, name=f"etn{tag}{qi}", tag=f"etn{qi}", bufs=2)
            nc.vector.tensor_scalar_mul(en[:qs], et[:qs], rt[:qs])
            e_t.append(et); etn.append(en); r_t.append(rt)
        # col sums over q (partitions) -> rc_row; P_T = e_t * bcast(rc_row)
        psr = ps.tile([1, LC], F32, tag="b", bufs=3)
        for qi, (qo, qs) in enumerate(QCH):
            nc.tensor.matmul(psr, ones_col[:qs, :1], e_t[qi][:qs],
                             start=(qi == 0), stop=(qi == 1))
        rc_row = po.tile([1, LC], F32, name=f"rc{tag}", tag="rc", bufs=2)
        nc.vector.reciprocal_approx_fast(rc_row, psr)
        rc_bf = po.tile([1, LC], BF, name=f"rcb{tag}", tag="rcb", bufs=2)
        nc.vector.tensor_copy(rc_bf, rc_row)
        P_T = []
        for qi, (qo, qs) in enumerate(QCH):
            pb = ps.tile([P, LC], F32, tag="a", bufs=4)
            nc.tensor.matmul(pb[:qs], ones_row[:1, :qs], rc_bf)
            pt = po.tile([P, LC], BF, name=f"PT{tag}{qi}", tag=f"PT{qi}", bufs=2)
            nc.vector.tensor_tensor(pt[:qs], e_t[qi][:qs], pb[:qs], op=MUL)
            P_T.append(pt)
        # etn_T [LC, LQ]: transpose e_t_norm
        etn_T = [po.tile([P, LQ], BF, name=f"etnT{tag}{c}", tag=f"etnT{c}", bufs=2)
                 for c in range(CC)]
        for c in range(CC):
            for qi, (qo, qs) in enumerate(QCH):
                pt = ps.tile([P, P], BF, tag="b", bufs=3)
                nc.tensor.transpose(pt[:, :qs], etn[qi][:qs, c * P:(c + 1) * P],
                                    ident[:qs, :qs])
                nc.vector.tensor_copy(etn_T[c][:, qo:qo + qs], pt[:, :qs])
        # tmp [LQ, D]
        tmp = []
        for qi, (qo, qs) in enumerate(QCH):
            t = po.tile([P, D], BF, name=f"tmp{tag}{qi}", tag=f"tmp{qi}", bufs=2)
            for n in range(2):
                pm = ps.tile([P, 384], F32, tag="a", bufs=4)
                for c in range(CC):
                    nc.tensor.matmul(pm[:qs], etn_T[c][:, qo:qo + qs],
                                     S_nat[:, c, n * 384:(n + 1) * 384],
                                     start=(c == 0), stop=(c == CC - 1))
                nc.scalar.activation(t[:qs, n * 384:(n + 1) * 384], pm[:qs], AF.Copy)
            tmp.append(t)
        # c2q_T, m1, m2 (the X4^T blocks beyond S_T and c2q_T itself)
        c2q_T = [po.tile([P, LC], BF, name=f"c2qT{tag}{d}", tag=f"c2qT{d}", bufs=1)
                 for d in range(DC)]
        m1 = [po.tile([P, LC], BF, name=f"m1{tag}{d}", tag=f"m1{d}", bufs=1)
              for d in range(DC)]
        m2 = [po.tile([P, LC], BF, name=f"m2{tag}{d}", tag=f"m2{d}", bufs=1)
              for d in range(DC)]
        for d in range(DC):
            pc = ps.tile([P, LC], F32, tag="a", bufs=4)
            for qi, (qo, qs) in enumerate(QCH):
                nc.tensor.matmul(pc, Qn[:qs, qi, d * P:(d + 1) * P], P_T[qi][:qs],
                                 start=(qi == 0), stop=(qi == 1))
            nc.scalar.activation(c2q_T[d], pc, AF.Copy)
            nc.gpsimd.tensor_tensor(m1[d], S_T[:, d, :], c2q_T[d], op=MUL)
            pq2 = ps.tile([P, LC], F32, tag="a", bufs=4)
            for qi, (qo, qs) in enumerate(QCH):
                nc.tensor.matmul(pq2, tmp[qi][:qs, d * P:(d + 1) * P], P_T[qi][:qs],
                                 start=(qi == 0), stop=(qi == 1))
            # m2 = S_T * q2c_T directly from psum
            nc.vector.tensor_tensor(m2[d], S_T[:, d, :], pq2, op=MUL)
        # cqa: out^T[dout, c] += cqa_WT-blocks
        xblocks = [S_T[:, d, :] for d in range(DC)] + \
                  [t[:] for t in c2q_T] + [t[:] for t in m1] + [t[:] for t in m2]
        for mc in range(DC):
            pco = ps.tile([P, LC], F32, tag="a", bufs=4)
            for k in range(4 * DC):
                nc.tensor.matmul(pco, cqa_slice(k, mc), xblocks[k],
                                 start=(k == 0), stop=(k == 4 * DC - 1))
            ob = att_T[row0 + mc]
            nc.scalar.activation(ob, pco, AF.Identity,
                                 bias=cqab[:, mc:mc + 1], scale=1.0)
            dma(out_t[(row0 + mc) * P:(row0 + mc + 1) * P, :], ob)

    s2q_call("q", 0)
    s2q_call("e", DC)
    s2qt.release(); qin.release(); cqaw.release(); s2q.release()

    # ---------------- phase 2: knowledge attention stack ----------------
    mp = tc.alloc_tile_pool(name="mp", bufs=1)
    lnr = []
    for l in range(NL):
        t = mp.tile([1, 4, D], BF, name=f"lnr{l}", tag=f"lnr{l}", bufs=1)
        dma(t, dt[f'ln{l}'].rearrange("(o d) v -> o v d", o=1))
        lnr.append(t)
    neg_row = mp.tile([1, LK], BF, name="neg_row", tag="neg_row", bufs=1)
    nc.gpsimd.memset(neg_row, -1.0)

    def proj_kv(kv_T, w, tag):
        """K^T and V projections; emittable early (cross-attn: inputs are att)."""
        nkv = len(kv_T)
        k_T = [mp.tile([P, LK], BF, name=f"k{tag}{m}", tag=f"kT{m}", bufs=2)
               for m in range(4)]
        for m in range(4):
            pss = ps.tile([P, LK], F32, tag="a", bufs=4)
            for k in range(nkv):
                nc.tensor.matmul(pss, w['k'](k, m), kv_T[k],
                                 start=(k == 0), stop=(k == nkv - 1))
            if m % 2 == 0:
                nc.vector.tensor_copy(k_T[m], pss)
            else:
                nc.scalar.activation(k_T[m], pss, AF.Copy)
        v_aug = [mp.tile([P, H, DV + 1], BF, name=f"va{tag}{c}", tag=f"va{c}",
                         bufs=2) for c in range(CC)]
        for c in range(CC):
            pvs = ps.tile([P, H * DV], F32, tag="a", bufs=4)
            for k in range(nkv):
                nc.tensor.matmul(pvs, kv_T[k][:, c * P:(c + 1) * P], w['v'](k),
                                 start=(k == 0), stop=(k == nkv - 1))
            nc.vector.tensor_copy(v_aug[c][:, :, 0:DV],
                                  pvs.rearrange("p (h d) -> p h d", h=H))
            nc.gpsimd.memset(v_aug[c][:, :, DV:DV + 1], 1.0)
        return k_T, v_aug

    def mha_ln(x_T, kv_T, w, g_ap, b_ap, tag, kv_pre=None, filler=None):
        """x_T: 6 [P,LK] bf16 query-side tiles; kv_T: 6 or 12 [P,LK] tiles.
        returns new 6 [P,LK] bf16 tiles = LN(fc(attn) + x_T)."""
        q_T = [mp.tile([P, LK], BF, name=f"q{tag}{m}", tag=f"qT{m}", bufs=1)
               for m in range(4)]
        for m in range(4):
            pss = ps.tile([P, LK], F32, tag="a", bufs=4)
            for k in range(DC):
                nc.tensor.matmul(pss, w['q'](k, m), x_T[k],
                                 start=(k == 0), stop=(k == DC - 1))
            if m % 2 == 0:
                nc.vector.tensor_copy(q_T[m], pss)
            else:
                nc.scalar.activation(q_T[m], pss, AF.Copy)
        if kv_pre is None:
            k_T, v_aug = proj_kv(kv_T, w, tag)
        else:
            k_T, v_aug = kv_pre
        # --- attention per head; denominators batched per 4-head block ---
        # head h parks its denominator row at partition 32*(h%4), col block h//4
        den = mp.tile([P, 2 * LK], F32, name=f"den{tag}", tag="den", bufs=1)
        nc.gpsimd.memset(den, 1.0)
        ovp = [mp.tile([P, LK], BF, name=f"ovp{tag}{t}", tag=f"ovp{t}", bufs=1)
               for t in range(4)]
        out_T = [mp.tile([P, LK], BF, name=f"o{tag}{m}", tag=f"oT{m}", bufs=1)
                 for m in range(4)]

        def finish_block(blk):
            # reciprocal + bf16 cast + broadcast/mult for heads 4*blk..4*blk+3
            denr = mp.tile([P, LK], F32, name=f"denr{tag}{blk}", tag=f"denr{blk}",
                           bufs=1)
            nc.vector.reciprocal_approx_fast(denr, den[:, blk * LK:(blk + 1) * LK])
            denb = mp.tile([P, LK], BF, name=f"denb{tag}{blk}", tag=f"denb{blk}",
                           bufs=1)
            nc.vector.tensor_copy(denb, denr)
            for tt in (2 * blk, 2 * blk + 1):
                pbc = ps.tile([P, LK], F32, tag="b", bufs=3)
                nc.tensor.matmul(pbc, selN[:, (tt % 2) * P:(tt % 2 + 1) * P], denb)
                nc.vector.tensor_tensor(out_T[tt], ovp[tt], pbc, op=MUL)

        for h in range(H):
            t, o = h // 2, (h % 2) * DK
            e_sb = []
            for c in range(CC):
                pa = ps.tile([P, LK], F32, tag="a", bufs=4)
                nc.tensor.matmul(pa, k_T[t][o:o + DK, c * P:(c + 1) * P],
                                 q_T[t][o:o + DK, :], start=True, stop=True)
                es = mp.tile([P, LK], BF, name=f"es{tag}{h}{c}", tag="es", bufs=6)
                nc.scalar.activation(es, pa, AF.Exp, scale=SCALE)
                e_sb.append(es)
            pov = ps.tile([DV + 1, LK], F32, tag="b", bufs=3)
            for c in range(CC):
                nc.tensor.matmul(pov, v_aug[c][:, h, :], e_sb[c],
                                 start=(c == 0), stop=(c == CC - 1))
            pr, sb = 32 * (h % 4), (h // 4) * LK
            nc.scalar.activation(den[pr:pr + 1, sb:sb + LK], pov[DV:DV + 1, :],
                                 AF.Copy)
            if h % 2 == 0:
                nc.vector.tensor_copy(ovp[t][0:DV, :], pov[:DV, :])
            else:
                nc.scalar.activation(ovp[t][DV:P, :], pov[:DV, :], AF.Copy)
            if h == 3:
                finish_block(0)
        finish_block(1)
        # --- fc + residual + LN ---
        x1 = [mp.tile([P, LK], BF, name=f"x1{tag}{d}", tag=f"x1{d}", bufs=1)
              for d in range(DC)]
        sqs = [mp.tile([P, LK], BF, name=f"sq{tag}{d}", tag="sq", bufs=3)
               for d in range(DC)]
        for d in range(DC):
            pf = ps.tile([P, LK], F32, tag="a", bufs=4)
            for k in range(4):
                nc.tensor.matmul(pf, w['fc'](k, d), out_T[k],
                                 start=(k == 0), stop=(k == 3))
            nc.vector.tensor_tensor(x1[d], pf, x_T[d], op=ADD)
            nc.gpsimd.tensor_tensor(sqs[d], x1[d], x1[d], op=MUL)
        # LN stats via ones-matmul over partitions
        ps_s = ps.tile([1, LK], F32, tag="b", bufs=3)
        ps_q = ps.tile([1, LK], F32, tag="b", bufs=3)
        for d in range(DC):
            nc.tensor.matmul(ps_s, ones_col, x1[d], start=(d == 0), stop=(d == DC - 1))
        for d in range(DC):
            nc.tensor.matmul(ps_q, ones_col, sqs[d], start=(d == 0), stop=(d == DC - 1))
        # independent matmul work to keep PE busy through the LN tail below
        filler_res = filler() if filler is not None else None
        mu = mp.tile([1, LK], F32, name=f"mu{tag}", tag="mu", bufs=1)
        nc.scalar.activation(mu, ps_s, AF.Copy, bias=0.0, scale=1.0 / D)
        msq = mp.tile([1, LK], F32, name=f"msq{tag}", tag="msq", bufs=1)
        nc.scalar.activation(msq, ps_q, AF.Copy, bias=0.0, scale=1.0 / D)
        var = mp.tile([1, LK], F32, name=f"var{tag}", tag="var", bufs=1)
        nc.vector.tensor_tensor(var, mu, mu, op=MUL)
        nc.vector.tensor_tensor(var, msq, var, op=SUB)
        # rstd = (var+eps)^-1/2 in one table op
        rstd = mp.tile([1, LK], BF, name=f"rstd{tag}", tag="rstd", bufs=1)
        nc.scalar.activation(rstd, var, AF.Abs_reciprocal_sqrt, bias=eps_t,
                             scale=1.0)
        c2 = mp.tile([1, LK], BF, name=f"c2{tag}", tag="c2", bufs=1)
        nc.vector.tensor_tensor(c2, mu, rstd, op=MUL)
        g_row, b_row = g_ap, b_ap       # [1, D] fp32 rows
        y = [mp.tile([P, LK], BF, name=f"y{tag}{d}", tag=f"y{tag[0]}{d}", bufs=1)
             for d in range(DC)]
        for d in range(DC):
            # pA = g_d (x) rstd ; pC = g_d (x) (mu*rstd) - b_d (rank-1 matmuls)
            pA = ps.tile([P, LK], F32, tag="b", bufs=3)
            nc.tensor.matmul(pA, g_row[:1, d * P:(d + 1) * P], rstd)
            pC = ps.tile([P, LK], F32, tag="b", bufs=3)
            nc.tensor.matmul(pC, g_row[:1, d * P:(d + 1) * P], c2,
                             start=True, stop=False)
            nc.tensor.matmul(pC, b_row[:1, d * P:(d + 1) * P], neg_row,
                             start=False, stop=True)
            pAb = mp.tile([P, LK], BF, name=f"pAb{tag}{d}", tag="pAb", bufs=2)
            nc.vector.tensor_copy(pAb, pA)
            pCb = mp.tile([P, LK], BF, name=f"pCb{tag}{d}", tag="pCb", bufs=2)
            nc.scalar.activation(pCb, pC, AF.Copy)
            if d % 2 == 0:
                nc.vector.tensor_tensor(y[d], x1[d], pAb, op=MUL)
                nc.vector.tensor_tensor(y[d], y[d], pCb, op=SUB)
            else:
                nc.gpsimd.tensor_tensor(y[d], x1[d], pAb, op=MUL)
                nc.gpsimd.tensor_tensor(y[d], y[d], pCb, op=SUB)
        return y, filler_res

    cur = [ke_T[:, d, :] for d in range(DC)]
    w_cur = {('s', 0): w_s0, ('c', 0): w_c0}
    for l in range(NL):
        g1, b1 = lnr[l][:, 0, :], lnr[l][:, 1, :]
        g2, b2 = lnr[l][:, 2, :], lnr[l][:, 3, :]
        if l + 1 < NL:
            w_cur[('s', l + 1)] = load_w(l + 1, True)
        # during the self-mha LN tail, compute this layer's cross K/V
        wc = w_cur[('c', l)]
        so, kv_c = mha_ln(cur, cur, w_cur[('s', l)], g1, b1, f"s{l}",
                          filler=(lambda wc=wc, l=l: proj_kv(att_T, wc, f"c{l}")))
        if l + 1 < NL:
            w_cur[('c', l + 1)] = load_w(l + 1, False)
        cur, _ = mha_ln(so, att_T, wc, g2, b2, f"c{l}", kv_pre=kv_c)
    for d in range(DC):
        dma(out_t[(2 * DC + d) * P:(2 * DC + d + 1) * P, :], cur[d])
    mp.release(); wpool.release()
    ctx.close()


def _pack_pairs(w):
    # [R, C] with R = 2k*128 -> [R/2, 2C]; row-chunk 2j at cols [0:C], 2j+1 at [C:2C]
    r, c = w.shape
    v = w.reshape(r // P // 2, 2, P, c)
    return np.ascontiguousarray(np.concatenate([v[:, 0], v[:, 1]], axis=2)
                                .reshape(r // 2, 2 * c))


def kernel(**inputs):
    if 'nc' not in _CACHE:
        _CACHE['nc'] = _build()
    nc = _CACHE['nc']
    f = lambda x: np.asarray(x, dtype=np.float32)
    b = lambda x: np.ascontiguousarray(np.asarray(x, dtype=np.float32).astype(bf16))

    def pad256(x):
        out = np.zeros((2 * P, D), dtype=np.float32)
        out[:x.shape[0]] = x
        return out

    seq = f(inputs['sequences']); qry = f(inputs['query']); evd = f(inputs['evidence'])
    ke = f(inputs['knowledge_embed'])
    vecs_b = np.ascontiguousarray(np.stack(
        [f(inputs['w4C'])[:, 0], f(inputs['w4Q'])[:, 0]], axis=1).astype(bf16))
    cqa_Wp = _pack_pairs(b(inputs['cqa_W']).T.copy())
    shared = {'vecs_b': vecs_b, 'w4mlu_f': f(inputs['w4mlu'])[0, 0, :].copy(),
              'cqa_b': f(inputs['cqa_b']), 'cqa_Wp': cqa_Wp}
    for l in range(NL):
        shared[f'sQKV{l}'] = _pack_pairs(np.concatenate(
            [b(inputs['L_sWq'][l]), b(inputs['L_sWk'][l]), b(inputs['L_sWv'][l])],
            axis=1))
        shared[f'sFC{l}'] = _pack_pairs(b(inputs['L_sWfc'][l]))
        shared[f'cQp{l}'] = _pack_pairs(b(inputs['L_cWq'][l]))
        shared[f'cKV{l}'] = _pack_pairs(np.concatenate(
            [b(inputs['L_cWk'][l]), b(inputs['L_cWv'][l])], axis=1))
        shared[f'cFC{l}'] = _pack_pairs(b(inputs['L_cWfc'][l]))
        shared[f'ln{l}'] = np.ascontiguousarray(np.stack(
            [f(inputs['L_n1g'][l]), f(inputs['L_n1b'][l]),
             f(inputs['L_n2g'][l]), f(inputs['L_n2b'][l])], axis=1).astype(bf16))
    in_maps = []
    for bi in range(B):
        m = {
            'S_nat': b(seq[bi]), 'S_T': b(seq[bi].T),
            'Q_nat': b(pad256(qry[bi])), 'Q_T': b(qry[bi].T),
            'E_nat': b(pad256(evd[bi])), 'E_T': b(evd[bi].T),
            'KE_T': b(ke[bi].T),
        }
        m.update(shared)
        in_maps.append(m)
    _CACHE['last_in_maps'] = in_maps
    res = run_bass_kernel_spmd(nc, in_maps, core_ids=list(range(B)))
    _CACHE['last_results'] = res
    outs = np.stack([r['out_t'].astype(np.float32) for r in res.results])
    out = np.concatenate([seq, outs.transpose(0, 2, 1)], axis=-1)
    return out
